# revision 4
# baseline (speedup 1.0000x reference)
"""AV-VQVAE encoder forward on 8 Trainium2 NeuronCores (Bass/Tile).

Strategy (data-parallel over batch B=128, 16 batches / 1024 tokens per core):
  - Codebook halves are matched against tokens with an exact-fp32 distance:
    B = x @ e.T computed as three fp16 matmuls (hi/lo split of both operands,
    scaled by powers of two, transposed on host) accumulated in fp32 PSUM
    -> ~1e-8 abs accuracy, preserving the reference's fp32 argmin decisions.
  - dist is assembled with the reference's exact rounding order:
    nd = fl(B*2^22 - fl(x2*2^21 + e2*2^21)) = -dist * 2^21 elementwise.
  - argmin via DVE top-8 max + index with first-index tie-break.
  - softmax(-sqrt(dist)) on ACT (sqrt, exp with per-row bias + row-sum accum),
    per-batch mean over T folded into a small PE matmul with 1/(64*zsum)
    weights, accumulated into pH.
  - quantized outputs gathered from the fp32 embedding with indirect DMA,
    straight-through arithmetic done in the reference's op order (bit-exact).
  - tiny finishers (Lcmcm scalar from pH, equal_num from indices) on host.
"""

import numpy as np

B, T, D, M = 128, 64, 1024, 2048
NCORES = 8
BT = B * T                  # 8192 tokens
NT = BT // NCORES           # 1024 tokens per core
NTT = NT // 128             # 8 token tiles per core
NDC = D // 128              # 8 contraction chunks
NMC = M // 512              # 4 moving chunks
XS = np.float32(2.0 ** 10)  # x split scale
ES = np.float32(2.0 ** 12)  # e split scale
S21 = float(2.0 ** 21)      # dist scale: B~ = 2B * 2^21 in PSUM

_CACHE = {}


def _split_waits(nc, max_waits=1):
    """Walrus in this toolchain allows only one sem-wait per instruction;
    move extras onto preceding same-engine nops (same-stream => identical
    semantics)."""
    import bass_rust
    import concourse.mybir as mybir

    ctr = 0
    for f in nc.m.functions:
        for blk in f.blocks:
            new_insts = []
            changed = False
            for inst in blk.instructions:
                si = inst.sync_info
                waits = list(si.on_wait) if si and si.on_wait else []
                if len(waits) > max_waits:
                    changed = True
                    extra, keep = waits[:-max_waits], waits[-max_waits:]
                    for i in range(0, len(extra), max_waits):
                        ctr += 1
                        nop = bass_rust.InstNoOp(
                            name=f"I-wsplit-{ctr}", ins=[], outs=[]
                        )
                        nop.engine = inst.engine
                        nop.sync_info = mybir.SyncInfo(
                            on_wait=extra[i : i + max_waits], on_update=[]
                        )
                        nc.register_instruction(nop)
                        new_insts.append(nop)
                    inst.sync_info = mybir.SyncInfo(
                        on_wait=keep,
                        on_update=list(si.on_update) if si.on_update else [],
                    )
                new_insts.append(inst)
            if changed:
                blk.instructions = new_insts


def _build():
    import concourse.bass as bass
    import concourse.mybir as mybir
    from concourse.tile import TileContext

    f32, f16, u32 = mybir.dt.float32, mybir.dt.float16, mybir.dt.uint32
    AF = mybir.ActivationFunctionType
    OP = mybir.AluOpType

    nc = bass.Bass("TRN2")

    d_video = nc.dram_tensor("video", [NT, D], f32, kind="ExternalInput")
    d_audio = nc.dram_tensor("audio", [NT, D], f32, kind="ExternalInput")
    d_emb = nc.dram_tensor("emb", [M, 2 * D], f32, kind="ExternalInput")
    # pre-transposed fp16 hi/lo splits (host): xT [D, NT], eT [D, M]
    d_xTh = {m: nc.dram_tensor(f"xTh_{m}", [D, NT], f16, kind="ExternalInput") for m in "va"}
    d_xTl = {m: nc.dram_tensor(f"xTl_{m}", [D, NT], f16, kind="ExternalInput") for m in "va"}
    d_eTh = {m: nc.dram_tensor(f"eTh_{m}", [D, M], f16, kind="ExternalInput") for m in "va"}
    d_eTl = {m: nc.dram_tensor(f"eTl_{m}", [D, M], f16, kind="ExternalInput") for m in "va"}
    d_e2s = {m: nc.dram_tensor(f"e2s_{m}", [1, M], f32, kind="ExternalInput") for m in "va"}

    d_full = {m: nc.dram_tensor(f"full_{m}", [NT, 2 * D], f32, kind="ExternalOutput") for m in "va"}
    d_vq = {m: nc.dram_tensor(f"vq_{m}", [NT, D], f32, kind="ExternalOutput") for m in "va"}
    d_idx = {m: nc.dram_tensor(f"idx_{m}", [NT, 1], u32, kind="ExternalOutput") for m in "va"}
    d_pH = {m: nc.dram_tensor(f"pH_{m}", [B // NCORES, M], f32, kind="ExternalOutput") for m in "va"}

    with TileContext(nc) as tc:
        with (
            tc.tile_pool(name="etab", bufs=1) as epool,     # eT splits + e2s bcast
            tc.tile_pool(name="work", bufs=2) as wpool,     # per-tile working tiles
            tc.tile_pool(name="ph", bufs=1) as hpool,       # pH accumulators
            tc.tile_pool(name="ps", bufs=8, space="PSUM") as ppool,
        ):
            for mod in "va":
                x_nat = d_video if mod == "v" else d_audio
                half = 0 if mod == "v" else D   # which half of the emb row for vq

                # --- modality tables: eT hi/lo [128, dc*2048], e2s broadcast ---
                eTh = epool.tile([128, NDC * M], f16, tag="eTh")
                eTl = epool.tile([128, NDC * M], f16, tag="eTl")
                nc.sync.dma_start(
                    eTh[:].rearrange("p (c m) -> p c m", c=NDC),
                    d_eTh[mod][:].rearrange("(c p) m -> p c m", p=128))
                nc.sync.dma_start(
                    eTl[:].rearrange("p (c m) -> p c m", c=NDC),
                    d_eTl[mod][:].rearrange("(c p) m -> p c m", p=128))
                e2sb = epool.tile([128, M], f32, tag="e2sb")
                nc.sync.dma_start(e2sb[:], d_e2s[mod][:].broadcast_to([128, M]))

                pH_sb = hpool.tile([B // NCORES, M], f32, tag="pH")
                nc.vector.memset(pH_sb[:], 0.0)

                for tt in range(NTT):
                    trow = slice(tt * 128, (tt + 1) * 128)

                    # token-tile inputs: xT splits [128, dc*128]
                    xTh = wpool.tile([128, D], f16, tag="xTh")
                    xTl = wpool.tile([128, D], f16, tag="xTl")
                    nc.sync.dma_start(
                        xTh[:].rearrange("p (c t) -> p c t", c=NDC),
                        d_xTh[mod][:, trow].rearrange("(c p) t -> p c t", p=128))
                    nc.sync.dma_start(
                        xTl[:].rearrange("p (c t) -> p c t", c=NDC),
                        d_xTl[mod][:, trow].rearrange("(c p) t -> p c t", p=128))
                    x_t = wpool.tile([128, D], f32, tag="x")
                    nc.sync.dma_start(x_t[:], x_nat[trow, :])

                    # x2s = fl(sum(x^2)) * 2^21 (exact power-2 scale)
                    sqs = wpool.tile([128, D], f32, tag="sqscratch")
                    x2r = wpool.tile([128, 1], f32, tag="x2r")
                    nc.scalar.activation(sqs[:], x_t[:], AF.Square, accum_out=x2r[:])
                    x2s = wpool.tile([128, 1], f32, tag="x2s")
                    nc.vector.tensor_scalar_mul(x2s[:], x2r[:], S21)

                    # S' = fl(e2s + x2s)  (matches reference rounding x2+e2, scaled)
                    Sp = wpool.tile([128, M], f32, tag="Sp")
                    nc.vector.tensor_scalar_add(Sp[:], e2sb[:], x2s[:])

                    # B~ = x' @ e'.T in 3 fp16 passes -> psum = 2B * 2^21
                    psB = [ppool.tile([128, 512], f32, tag="ps", name=f"psB{_mc}")
                           for _mc in range(NMC)]
                    for dc in range(NDC):
                        for pi, (xs_t, es_t) in enumerate(((xTh, eTh), (xTh, eTl), (xTl, eTh))):
                            lhs = xs_t[:, dc * 128 : (dc + 1) * 128]
                            for mc in range(NMC):
                                nc.tensor.matmul(
                                    psB[mc][:], lhs,
                                    es_t[:, dc * M + mc * 512 : dc * M + (mc + 1) * 512],
                                    start=(dc == 0 and pi == 0),
                                    stop=(dc == NDC - 1 and pi == 2),
                                )

                    # nd = fl(B~ - S') = -dist * 2^21 elementwise
                    nd = wpool.tile([128, M], f32, tag="nd")
                    for mc in range(NMC):
                        nc.vector.tensor_tensor(
                            out=nd[:, mc * 512 : (mc + 1) * 512], in0=psB[mc][:],
                            in1=Sp[:, mc * 512 : (mc + 1) * 512], op=OP.subtract)

                    # argmax(nd) with first-index tie-break == argmin(dist)
                    m8 = wpool.tile([128, 8], f32, tag="m8")
                    i8 = wpool.tile([128, 8], u32, tag="i8")
                    nc.vector.max_with_indices(m8[:], i8[:], nd[:])
                    i8f = wpool.tile([128, 8], f32, tag="i8f")
                    nc.vector.tensor_copy(i8f[:], i8[:])
                    maskf = wpool.tile([128, 8], f32, tag="maskf")
                    nc.vector.tensor_scalar(maskf[:], m8[:], m8[:, 0:1], None, op0=OP.is_equal)
                    nm = wpool.tile([128, 8], f32, tag="nm")
                    nc.vector.tensor_scalar(nm[:], maskf[:], -4096.0, 4096.0, op0=OP.mult, op1=OP.add)
                    nc.vector.tensor_tensor(out=i8f[:], in0=i8f[:], in1=nm[:], op=OP.add)
                    vidxf = wpool.tile([128, 1], f32, tag="vidxf")
                    nc.vector.tensor_reduce(vidxf[:], i8f[:], axis=mybir.AxisListType.X, op=OP.min)
                    vidx = wpool.tile([128, 1], u32, tag="vidx")
                    nc.vector.tensor_copy(vidx[:], vidxf[:])
                    nc.sync.dma_start(d_idx[mod][trow, :], vidx[:])

                    # sqrt(dist) and softmax numerator/denominator on ACT
                    bias_s = wpool.tile([128, 1], f32, tag="bias")
                    nc.scalar.activation(bias_s[:], m8[:, 0:1], AF.Sqrt, scale=-float(2.0 ** -21))
                    sq = wpool.tile([128, M], f32, tag="sq")
                    nc.scalar.activation(sq[:], nd[:], AF.Sqrt, scale=-float(2.0 ** -21))
                    z = wpool.tile([128, M], f16, tag="z")
                    zsum = wpool.tile([128, 1], f32, tag="zsum")
                    nc.scalar.activation(z[:], sq[:], AF.Exp, bias=bias_s[:], scale=-1.0,
                                         accum_out=zsum[:])

                    # pH += A.T @ z with A[t, b] = 1/(64*zsum_t) on this tile's 2 batches
                    rcp = wpool.tile([128, 1], f32, tag="rcp")
                    nc.vector.reciprocal(rcp[:], zsum[:])
                    rs = wpool.tile([128, 1], f32, tag="rs")
                    nc.vector.tensor_scalar_mul(rs[:], rcp[:], float(2.0 ** -6))
                    A_t = wpool.tile([128, 16], f16, tag="A")
                    nc.vector.memset(A_t[:], 0.0)
                    nc.vector.tensor_copy(A_t[0:64, 2 * tt : 2 * tt + 1], rs[0:64, :])
                    nc.vector.tensor_copy(A_t[64:128, 2 * tt + 1 : 2 * tt + 2], rs[64:128, :])
                    for mc in range(NMC):
                        psP = ppool.tile([16, 512], f32, tag="ps", name=f"psP{mc}")
                        nc.tensor.matmul(psP[:], A_t[:], z[:, mc * 512 : (mc + 1) * 512],
                                         start=True, stop=True)
                        nc.vector.tensor_tensor(
                            out=pH_sb[:, mc * 512 : (mc + 1) * 512],
                            in0=pH_sb[:, mc * 512 : (mc + 1) * 512], in1=psP[:], op=OP.add)

                    # gather full codebook rows; straight-through vq in ref op order
                    g = wpool.tile([128, 2 * D], f32, tag="g")
                    nc.gpsimd.indirect_dma_start(
                        out=g[:], out_offset=None, in_=d_emb[:],
                        in_offset=bass.IndirectOffsetOnAxis(ap=vidx[:, :1], axis=0))
                    vq = wpool.tile([128, D], f32, tag="vq")
                    nc.vector.tensor_tensor(out=vq[:], in0=g[:, half : half + D], in1=x_t[:], op=OP.subtract)
                    nc.vector.tensor_tensor(out=vq[:], in0=vq[:], in1=x_t[:], op=OP.add)
                    nc.sync.dma_start(d_full[mod][trow, :], g[:])
                    nc.sync.dma_start(d_vq[mod][trow, :], vq[:])

                nc.sync.dma_start(d_pH[mod][:], pH_sb[:])

    _split_waits(nc, max_waits=1)
    return nc


def _prep_host(audio, video, embedding):
    """Host-side prep: per-core slices, scaled fp16 hi/lo splits (transposed),
    e2 row sums."""
    a = np.ascontiguousarray(np.asarray(audio, dtype=np.float32).reshape(BT, D))
    v = np.ascontiguousarray(np.asarray(video, dtype=np.float32).reshape(BT, D))
    emb = np.ascontiguousarray(np.asarray(embedding, dtype=np.float32))

    def split_T(x, s):
        xs = x * s
        h = xs.astype(np.float16)
        l = (xs - h.astype(np.float32)).astype(np.float16)
        return np.ascontiguousarray(h.T), np.ascontiguousarray(l.T)

    vTh, vTl = split_T(v, XS)       # [D, BT]
    aTh, aTl = split_T(a, XS)
    e_v, e_a = emb[:, :D], emb[:, D:]
    evTh, evTl = split_T(e_v, ES)   # [D, M]
    eaTh, eaTl = split_T(e_a, ES)
    e2s_v = ((e_v * e_v).sum(axis=1, dtype=np.float32).astype(np.float32)
             * np.float32(S21)).reshape(1, M)
    e2s_a = ((e_a * e_a).sum(axis=1, dtype=np.float32).astype(np.float32)
             * np.float32(S21)).reshape(1, M)

    in_maps = []
    for c in range(NCORES):
        r = slice(c * NT, (c + 1) * NT)
        in_maps.append({
            "video": v[r], "audio": a[r], "emb": emb,
            "xTh_v": np.ascontiguousarray(vTh[:, r]),
            "xTl_v": np.ascontiguousarray(vTl[:, r]),
            "xTh_a": np.ascontiguousarray(aTh[:, r]),
            "xTl_a": np.ascontiguousarray(aTl[:, r]),
            "eTh_v": evTh, "eTl_v": evTl, "eTh_a": eaTh, "eTl_a": eaTl,
            "e2s_v": e2s_v, "e2s_a": e2s_a,
        })
    return in_maps, (a, v, emb)


def _finish_host(results, a, v, emb):
    """Assemble per-core outputs; compute Lcmcm + equal_num on host (fp32)."""
    full_v = np.concatenate([r["full_v"] for r in results], axis=0)
    full_a = np.concatenate([r["full_a"] for r in results], axis=0)
    vq_v = np.concatenate([r["vq_v"] for r in results], axis=0)
    vq_a = np.concatenate([r["vq_a"] for r in results], axis=0)
    idx_v = np.concatenate([r["idx_v"] for r in results], axis=0)[:, 0].astype(np.int64)
    idx_a = np.concatenate([r["idx_a"] for r in results], axis=0)[:, 0].astype(np.int64)
    pH_v = np.concatenate([r["pH_v"] for r in results], axis=0)  # [B, M]
    pH_a = np.concatenate([r["pH_a"] for r in results], axis=0)

    eps = np.float32(1e-5)
    logv = np.log(pH_v.T + np.float32(1e-10), dtype=np.float32)
    loga = np.log(pH_a.T + np.float32(1e-10), dtype=np.float32)
    Scode = pH_a @ logv + pH_v @ loga
    MaxScode = np.max(-Scode)
    EScode = np.exp(Scode + MaxScode, dtype=np.float32)
    ES1 = EScode.sum(axis=1, dtype=np.float32)
    Lcmcm = np.float32(-np.mean(np.log(np.diagonal(EScode) / (ES1 + eps),
                                       dtype=np.float32), dtype=np.float32))

    iv = idx_v.reshape(B, T)
    ia = idx_a.reshape(B, T)
    cv = np.zeros((B, M), np.int32)
    ca = np.zeros((B, M), np.int32)
    rows = np.repeat(np.arange(B), T)
    np.add.at(cv, (rows, iv.reshape(-1)), 1)
    np.add.at(ca, (rows, ia.reshape(-1)), 1)
    equal_num = np.int32((cv.argmax(axis=1) == ca.argmax(axis=1)).sum())

    return (
        full_v.reshape(B, T, 2 * D), full_a.reshape(B, T, 2 * D),
        vq_v.reshape(B, T, D), vq_a.reshape(B, T, D),
        Lcmcm, equal_num,
    )


def kernel(audio_semantic, video_semantic, embedding, modality_weights,
           hierarchical_weights, epoch):
    from concourse.bass_utils import run_bass_kernel_spmd

    if "nc" not in _CACHE:
        _CACHE["nc"] = _build()
    nc = _CACHE["nc"]

    in_maps, (a, v, emb) = _prep_host(audio_semantic, video_semantic, embedding)
    res = run_bass_kernel_spmd(nc, in_maps, core_ids=list(range(NCORES)))
    return _finish_host(res.results, a, v, emb)


# revision 7
# speedup vs baseline: 1.1369x; 1.1369x over previous
"""AV-VQVAE encoder forward on 8 Trainium2 NeuronCores (Bass/Tile).

Strategy (data-parallel over batch B=128, 16 batches / 1024 tokens per core):
  - Codebook halves are matched against tokens with an exact-fp32 distance:
    B = x @ e.T computed as three fp16 matmuls (hi/lo split of both operands,
    scaled by powers of two, transposed on host) accumulated in fp32 PSUM
    -> ~1e-8 abs accuracy, preserving the reference's fp32 argmin decisions.
  - dist is assembled with the reference's exact rounding order:
    nd = fl(B*2^22 - fl(x2*2^21 + e2*2^21)) = -dist * 2^21 elementwise.
  - argmin via DVE top-8 max + index with first-index tie-break.
  - softmax(-sqrt(dist)) on ACT (sqrt, exp with per-row bias + row-sum accum),
    per-batch mean over T folded into a small PE matmul with 1/(64*zsum)
    weights, accumulated into pH.
  - quantized outputs gathered from the fp32 embedding with indirect DMA,
    straight-through arithmetic done in the reference's op order (bit-exact).
  - tiny finishers (Lcmcm scalar from pH, equal_num from indices) on host.
"""

import numpy as np

B, T, D, M = 128, 64, 1024, 2048
NCORES = 8
BT = B * T                  # 8192 tokens
NT = BT // NCORES           # 1024 tokens per core
NTT = NT // 128             # 8 token tiles per core
NDC = D // 128              # 8 contraction chunks
NMC = M // 512              # 4 moving chunks
XS = np.float32(2.0 ** 10)  # x split scale
ES = np.float32(2.0 ** 12)  # e split scale
S21 = float(2.0 ** 21)      # dist scale: B~ = 2B * 2^21 in PSUM

_CACHE = {}


def _split_waits(nc, max_waits=1):
    """Walrus in this toolchain allows only one sem-wait per instruction;
    move extras onto preceding same-engine nops (same-stream => identical
    semantics)."""
    import bass_rust
    import concourse.mybir as mybir

    ctr = 0
    for f in nc.m.functions:
        for blk in f.blocks:
            new_insts = []
            changed = False
            for inst in blk.instructions:
                si = inst.sync_info
                waits = list(si.on_wait) if si and si.on_wait else []
                if len(waits) > max_waits:
                    changed = True
                    extra, keep = waits[:-max_waits], waits[-max_waits:]
                    for i in range(0, len(extra), max_waits):
                        ctr += 1
                        nop = bass_rust.InstNoOp(
                            name=f"I-wsplit-{ctr}", ins=[], outs=[]
                        )
                        nop.engine = inst.engine
                        nop.sync_info = mybir.SyncInfo(
                            on_wait=extra[i : i + max_waits], on_update=[]
                        )
                        nc.register_instruction(nop)
                        new_insts.append(nop)
                    inst.sync_info = mybir.SyncInfo(
                        on_wait=keep,
                        on_update=list(si.on_update) if si.on_update else [],
                    )
                new_insts.append(inst)
            if changed:
                blk.instructions = new_insts


def _build():
    import concourse.bass as bass
    import concourse.mybir as mybir
    from concourse.tile import TileContext

    f32, f16, u32 = mybir.dt.float32, mybir.dt.float16, mybir.dt.uint32
    AF = mybir.ActivationFunctionType
    OP = mybir.AluOpType

    nc = bass.Bass("TRN2")

    d_video = nc.dram_tensor("video", [NT, D], f32, kind="ExternalInput")
    d_audio = nc.dram_tensor("audio", [NT, D], f32, kind="ExternalInput")
    d_emb = nc.dram_tensor("emb", [M, 2 * D], f32, kind="ExternalInput")
    # pre-transposed fp16 hi/lo splits (host): xT [D, NT], eT [D, M]
    d_xTh = {m: nc.dram_tensor(f"xTh_{m}", [D, NT], f16, kind="ExternalInput") for m in "va"}
    d_xTl = {m: nc.dram_tensor(f"xTl_{m}", [D, NT], f16, kind="ExternalInput") for m in "va"}
    d_eTh = {m: nc.dram_tensor(f"eTh_{m}", [D, M], f16, kind="ExternalInput") for m in "va"}
    d_eTl = {m: nc.dram_tensor(f"eTl_{m}", [D, M], f16, kind="ExternalInput") for m in "va"}
    d_e2s = {m: nc.dram_tensor(f"e2s_{m}", [1, M], f32, kind="ExternalInput") for m in "va"}

    d_full = {m: nc.dram_tensor(f"full_{m}", [NT, 2 * D], f32, kind="ExternalOutput") for m in "va"}
    d_vq = {m: nc.dram_tensor(f"vq_{m}", [NT, D], f32, kind="ExternalOutput") for m in "va"}
    d_idx = {m: nc.dram_tensor(f"idx_{m}", [NT, 1], u32, kind="ExternalOutput") for m in "va"}
    d_pH = {m: nc.dram_tensor(f"pH_{m}", [B // NCORES, M], f32, kind="ExternalOutput") for m in "va"}

    with TileContext(nc) as tc:
        with (
            tc.tile_pool(name="etab", bufs=1) as epool,     # eT splits + e2s bcast
            tc.tile_pool(name="work", bufs=2) as wpool,     # per-tile working tiles
            tc.tile_pool(name="ph", bufs=1) as hpool,       # pH accumulators
            tc.tile_pool(name="zbuf", bufs=1) as zpool,     # per-tile z/A kept for pH phase
            tc.tile_pool(name="sp", bufs=1) as spool,       # S' (single-buffered)
            tc.tile_pool(name="ps", bufs=8, space="PSUM") as ppool,
        ):
            for mod in "va":
                x_nat = d_video if mod == "v" else d_audio
                half = 0 if mod == "v" else D   # which half of the emb row for vq

                # --- modality tables: eT hi/lo [128, dc*2048], e2s broadcast ---
                eTh = epool.tile([128, NDC * M], f16, tag="eTh")
                eTl = epool.tile([128, NDC * M], f16, tag="eTl")
                for dc in range(NDC):
                    nc.sync.dma_start(eTh[:, dc * M : (dc + 1) * M],
                                      d_eTh[mod][dc * 128 : (dc + 1) * 128, :])
                    nc.sync.dma_start(eTl[:, dc * M : (dc + 1) * M],
                                      d_eTl[mod][dc * 128 : (dc + 1) * 128, :])
                e2sb = epool.tile([128, M], f32, tag="e2sb")
                nc.sync.dma_start(e2sb[:], d_e2s[mod][:].broadcast_to([128, M]))

                pH_sb = hpool.tile([B // NCORES, M], f32, tag="pH")
                nc.vector.memset(pH_sb[:], 0.0)
                z_tiles, A_tiles = [], []

                for tt in range(NTT):
                    trow = slice(tt * 128, (tt + 1) * 128)

                    # token-tile inputs: xT splits [128, dc*128]
                    xTh = wpool.tile([128, D], f16, tag="xTh")
                    xTl = wpool.tile([128, D], f16, tag="xTl")
                    nc.sync.dma_start(
                        xTh[:].rearrange("p (c t) -> p c t", c=NDC),
                        d_xTh[mod][:, trow].rearrange("(c p) t -> p c t", p=128))
                    nc.sync.dma_start(
                        xTl[:].rearrange("p (c t) -> p c t", c=NDC),
                        d_xTl[mod][:, trow].rearrange("(c p) t -> p c t", p=128))
                    x_t = wpool.tile([128, D], f32, tag="x")
                    nc.sync.dma_start(x_t[:], x_nat[trow, :])

                    # x2s = fl(sum(x^2)) * 2^21 (exact power-2 scale)
                    sqs = wpool.tile([128, D], f16, tag="sqscratch")
                    x2r = wpool.tile([128, 1], f32, tag="x2r")
                    nc.scalar.activation(sqs[:], x_t[:], AF.Square, accum_out=x2r[:])
                    x2s = wpool.tile([128, 1], f32, tag="x2s")
                    nc.vector.tensor_scalar_mul(x2s[:], x2r[:], S21)

                    # S' = fl(e2s + x2s)  (matches reference rounding x2+e2, scaled)
                    Sp = spool.tile([128, M], f32, tag="Sp")
                    nc.vector.tensor_scalar_add(Sp[:], e2sb[:], x2s[:])

                    # B~ = x' @ e'.T in 3 fp16 passes -> psum = 2B * 2^21
                    psB = [ppool.tile([128, 512], f32, tag="ps", name=f"psB{_mc}")
                           for _mc in range(NMC)]
                    for dc in range(NDC):
                        for pi, (xs_t, es_t) in enumerate(((xTh, eTh), (xTh, eTl), (xTl, eTh))):
                            lhs = xs_t[:, dc * 128 : (dc + 1) * 128]
                            for mc in range(NMC):
                                nc.tensor.matmul(
                                    psB[mc][:], lhs,
                                    es_t[:, dc * M + mc * 512 : dc * M + (mc + 1) * 512],
                                    start=(dc == 0 and pi == 0),
                                    stop=(dc == NDC - 1 and pi == 2),
                                )

                    # nd = fl(B~ - S') = -dist * 2^21 elementwise
                    nd = wpool.tile([128, M], f32, tag="nd")
                    for mc in range(NMC):
                        nc.vector.tensor_tensor(
                            out=nd[:, mc * 512 : (mc + 1) * 512], in0=psB[mc][:],
                            in1=Sp[:, mc * 512 : (mc + 1) * 512], op=OP.subtract)

                    # argmax(nd) with first-index tie-break == argmin(dist)
                    m8 = wpool.tile([128, 8], f32, tag="m8")
                    i8 = wpool.tile([128, 8], u32, tag="i8")
                    nc.vector.max_with_indices(m8[:], i8[:], nd[:])
                    i8f = wpool.tile([128, 8], f32, tag="i8f")
                    nc.vector.tensor_copy(i8f[:], i8[:])
                    maskf = wpool.tile([128, 8], f32, tag="maskf")
                    nc.vector.tensor_scalar(maskf[:], m8[:], m8[:, 0:1], None, op0=OP.is_equal)
                    nm = wpool.tile([128, 8], f32, tag="nm")
                    nc.vector.tensor_scalar(nm[:], maskf[:], -4096.0, 4096.0, op0=OP.mult, op1=OP.add)
                    nc.vector.tensor_tensor(out=i8f[:], in0=i8f[:], in1=nm[:], op=OP.add)
                    vidxf = wpool.tile([128, 1], f32, tag="vidxf")
                    nc.vector.tensor_reduce(vidxf[:], i8f[:], axis=mybir.AxisListType.X, op=OP.min)
                    vidx = wpool.tile([128, 1], u32, tag="vidx")
                    nc.vector.tensor_copy(vidx[:], vidxf[:])
                    nc.sync.dma_start(d_idx[mod][trow, :], vidx[:])

                    # sqrt(dist) and softmax numerator/denominator on ACT
                    bias_s = wpool.tile([128, 1], f32, tag="bias")
                    nc.scalar.activation(bias_s[:], m8[:, 0:1], AF.Sqrt, scale=-float(2.0 ** -21))
                    sq = wpool.tile([128, M], f32, tag="sq")
                    nc.scalar.activation(sq[:], nd[:], AF.Sqrt, scale=-float(2.0 ** -21))
                    z = zpool.tile([128, M], f16, tag=f"z{tt}")
                    zsum = wpool.tile([128, 1], f32, tag="zsum")
                    nc.scalar.activation(z[:], sq[:], AF.Exp, bias=bias_s[:], scale=-1.0,
                                         accum_out=zsum[:])
                    z_tiles.append(z)

                    # A[t, b] = 1/(64*zsum_t) on this tile's 2 batches (for deferred pH)
                    rcp = wpool.tile([128, 1], f32, tag="rcp")
                    nc.vector.reciprocal(rcp[:], zsum[:])
                    rs = wpool.tile([128, 1], f32, tag="rs")
                    nc.vector.tensor_scalar_mul(rs[:], rcp[:], float(2.0 ** -6))
                    A_t = zpool.tile([128, 16], f16, tag=f"A{tt}")
                    nc.vector.memset(A_t[:], 0.0)
                    nc.vector.tensor_copy(A_t[0:64, 2 * tt : 2 * tt + 1], rs[0:64, :])
                    nc.vector.tensor_copy(A_t[64:128, 2 * tt + 1 : 2 * tt + 2], rs[64:128, :])
                    A_tiles.append(A_t)

                    # gather full codebook rows; straight-through vq in ref op order
                    g = wpool.tile([128, 2 * D], f32, tag="g")
                    nc.gpsimd.indirect_dma_start(
                        out=g[:], out_offset=None, in_=d_emb[:],
                        in_offset=bass.IndirectOffsetOnAxis(ap=vidx[:, :1], axis=0))
                    vq = wpool.tile([128, D], f32, tag="vq")
                    nc.vector.tensor_tensor(out=vq[:], in0=g[:, half : half + D], in1=x_t[:], op=OP.subtract)
                    nc.vector.tensor_tensor(out=vq[:], in0=vq[:], in1=x_t[:], op=OP.add)
                    nc.sync.dma_start(d_full[mod][trow, :], g[:])
                    nc.sync.dma_start(d_vq[mod][trow, :], vq[:])

                for tt in range(NTT):
                    for mc in range(NMC):
                        psP = ppool.tile([16, 512], f32, tag="ps", name=f"psP{tt}_{mc}")
                        nc.tensor.matmul(psP[:], A_tiles[tt][:],
                                         z_tiles[tt][:, mc * 512 : (mc + 1) * 512],
                                         start=True, stop=True)
                        nc.vector.tensor_tensor(
                            out=pH_sb[:, mc * 512 : (mc + 1) * 512],
                            in0=pH_sb[:, mc * 512 : (mc + 1) * 512], in1=psP[:], op=OP.add)
                nc.sync.dma_start(d_pH[mod][:], pH_sb[:])

    _split_waits(nc, max_waits=1)
    return nc


def _prep_host(audio, video, embedding):
    """Host-side prep: per-core slices, scaled fp16 hi/lo splits (transposed),
    e2 row sums."""
    a = np.ascontiguousarray(np.asarray(audio, dtype=np.float32).reshape(BT, D))
    v = np.ascontiguousarray(np.asarray(video, dtype=np.float32).reshape(BT, D))
    emb = np.ascontiguousarray(np.asarray(embedding, dtype=np.float32))

    def split_T(x, s):
        xs = x * s
        h = xs.astype(np.float16)
        l = (xs - h.astype(np.float32)).astype(np.float16)
        return np.ascontiguousarray(h.T), np.ascontiguousarray(l.T)

    vTh, vTl = split_T(v, XS)       # [D, BT]
    aTh, aTl = split_T(a, XS)
    e_v, e_a = emb[:, :D], emb[:, D:]
    evTh, evTl = split_T(e_v, ES)   # [D, M]
    eaTh, eaTl = split_T(e_a, ES)
    e2s_v = ((e_v * e_v).sum(axis=1, dtype=np.float32).astype(np.float32)
             * np.float32(S21)).reshape(1, M)
    e2s_a = ((e_a * e_a).sum(axis=1, dtype=np.float32).astype(np.float32)
             * np.float32(S21)).reshape(1, M)

    in_maps = []
    for c in range(NCORES):
        r = slice(c * NT, (c + 1) * NT)
        in_maps.append({
            "video": v[r], "audio": a[r], "emb": emb,
            "xTh_v": np.ascontiguousarray(vTh[:, r]),
            "xTl_v": np.ascontiguousarray(vTl[:, r]),
            "xTh_a": np.ascontiguousarray(aTh[:, r]),
            "xTl_a": np.ascontiguousarray(aTl[:, r]),
            "eTh_v": evTh, "eTl_v": evTl, "eTh_a": eaTh, "eTl_a": eaTl,
            "e2s_v": e2s_v, "e2s_a": e2s_a,
        })
    return in_maps, (a, v, emb)


def _finish_host(results, a, v, emb):
    """Assemble per-core outputs; compute Lcmcm + equal_num on host (fp32)."""
    full_v = np.concatenate([r["full_v"] for r in results], axis=0)
    full_a = np.concatenate([r["full_a"] for r in results], axis=0)
    vq_v = np.concatenate([r["vq_v"] for r in results], axis=0)
    vq_a = np.concatenate([r["vq_a"] for r in results], axis=0)
    idx_v = np.concatenate([r["idx_v"] for r in results], axis=0)[:, 0].astype(np.int64)
    idx_a = np.concatenate([r["idx_a"] for r in results], axis=0)[:, 0].astype(np.int64)
    pH_v = np.concatenate([r["pH_v"] for r in results], axis=0)  # [B, M]
    pH_a = np.concatenate([r["pH_a"] for r in results], axis=0)

    eps = np.float32(1e-5)
    logv = np.log(pH_v.T + np.float32(1e-10), dtype=np.float32)
    loga = np.log(pH_a.T + np.float32(1e-10), dtype=np.float32)
    Scode = pH_a @ logv + pH_v @ loga
    MaxScode = np.max(-Scode)
    EScode = np.exp(Scode + MaxScode, dtype=np.float32)
    ES1 = EScode.sum(axis=1, dtype=np.float32)
    Lcmcm = np.float32(-np.mean(np.log(np.diagonal(EScode) / (ES1 + eps),
                                       dtype=np.float32), dtype=np.float32))

    iv = idx_v.reshape(B, T)
    ia = idx_a.reshape(B, T)
    cv = np.zeros((B, M), np.int32)
    ca = np.zeros((B, M), np.int32)
    rows = np.repeat(np.arange(B), T)
    np.add.at(cv, (rows, iv.reshape(-1)), 1)
    np.add.at(ca, (rows, ia.reshape(-1)), 1)
    equal_num = np.int32((cv.argmax(axis=1) == ca.argmax(axis=1)).sum())

    return (
        full_v.reshape(B, T, 2 * D), full_a.reshape(B, T, 2 * D),
        vq_v.reshape(B, T, D), vq_a.reshape(B, T, D),
        Lcmcm, equal_num,
    )


def kernel(audio_semantic, video_semantic, embedding, modality_weights,
           hierarchical_weights, epoch):
    from concourse.bass_utils import run_bass_kernel_spmd

    if "nc" not in _CACHE:
        _CACHE["nc"] = _build()
    nc = _CACHE["nc"]

    in_maps, (a, v, emb) = _prep_host(audio_semantic, video_semantic, embedding)
    res = run_bass_kernel_spmd(nc, in_maps, core_ids=list(range(NCORES)))
    return _finish_host(res.results, a, v, emb)


# revision 8
# speedup vs baseline: 1.1870x; 1.0441x over previous
"""AV-VQVAE encoder forward on 8 Trainium2 NeuronCores (Bass/Tile).

Strategy (data-parallel over batch B=128, 16 batches / 1024 tokens per core):
  - Codebook halves are matched against tokens with an exact-fp32 distance:
    B = x @ e.T computed as three fp16 matmuls (hi/lo split of both operands,
    scaled by powers of two, transposed on host) accumulated in fp32 PSUM
    -> ~1e-8 abs accuracy, preserving the reference's fp32 argmin decisions.
  - dist is assembled with the reference's exact rounding order:
    nd = fl(B*2^22 - fl(x2*2^21 + e2*2^21)) = -dist * 2^21 elementwise.
  - argmin via DVE top-8 max + index with first-index tie-break.
  - softmax(-sqrt(dist)) on ACT (sqrt, exp with per-row bias + row-sum accum),
    per-batch mean over T folded into a small PE matmul with 1/(64*zsum)
    weights, accumulated into pH.
  - quantized outputs gathered from the fp32 embedding with indirect DMA,
    straight-through arithmetic done in the reference's op order (bit-exact).
  - tiny finishers (Lcmcm scalar from pH, equal_num from indices) on host.
"""

import numpy as np

B, T, D, M = 128, 64, 1024, 2048
NCORES = 8
BT = B * T                  # 8192 tokens
NT = BT // NCORES           # 1024 tokens per core
NTT = NT // 128             # 8 token tiles per core
NDC = D // 128              # 8 contraction chunks
NMC = M // 512              # 4 moving chunks
XS = np.float32(2.0 ** 10)  # x split scale
ES = np.float32(2.0 ** 12)  # e split scale
S21 = float(2.0 ** 21)      # dist scale: B~ = 2B * 2^21 in PSUM

_CACHE = {}


def _split_waits(nc, max_waits=1):
    """Walrus in this toolchain allows only one sem-wait per instruction;
    move extras onto preceding same-engine nops (same-stream => identical
    semantics)."""
    import bass_rust
    import concourse.mybir as mybir

    ctr = 0
    for f in nc.m.functions:
        for blk in f.blocks:
            new_insts = []
            changed = False
            for inst in blk.instructions:
                si = inst.sync_info
                waits = list(si.on_wait) if si and si.on_wait else []
                if len(waits) > max_waits:
                    changed = True
                    extra, keep = waits[:-max_waits], waits[-max_waits:]
                    for i in range(0, len(extra), max_waits):
                        ctr += 1
                        nop = bass_rust.InstNoOp(
                            name=f"I-wsplit-{ctr}", ins=[], outs=[]
                        )
                        nop.engine = inst.engine
                        nop.sync_info = mybir.SyncInfo(
                            on_wait=extra[i : i + max_waits], on_update=[]
                        )
                        nc.register_instruction(nop)
                        new_insts.append(nop)
                    inst.sync_info = mybir.SyncInfo(
                        on_wait=keep,
                        on_update=list(si.on_update) if si.on_update else [],
                    )
                new_insts.append(inst)
            if changed:
                blk.instructions = new_insts


def _build():
    import concourse.bass as bass
    import concourse.mybir as mybir
    from concourse.tile import TileContext

    f32, f16, u32 = mybir.dt.float32, mybir.dt.float16, mybir.dt.uint32
    AF = mybir.ActivationFunctionType
    OP = mybir.AluOpType

    nc = bass.Bass("TRN2")

    d_video = nc.dram_tensor("video", [NT, D], f32, kind="ExternalInput")
    d_audio = nc.dram_tensor("audio", [NT, D], f32, kind="ExternalInput")
    d_emb = nc.dram_tensor("emb", [M, 2 * D], f32, kind="ExternalInput")
    # pre-transposed fp16 hi/lo splits (host): xT [D, NT], eT [D, M]
    d_xTh = {m: nc.dram_tensor(f"xTh_{m}", [D, NT], f16, kind="ExternalInput") for m in "va"}
    d_xTl = {m: nc.dram_tensor(f"xTl_{m}", [D, NT], f16, kind="ExternalInput") for m in "va"}
    d_eTh = {m: nc.dram_tensor(f"eTh_{m}", [D, M], f16, kind="ExternalInput") for m in "va"}
    d_eTl = {m: nc.dram_tensor(f"eTl_{m}", [D, M], f16, kind="ExternalInput") for m in "va"}
    d_e2s = {m: nc.dram_tensor(f"e2s_{m}", [1, M], f32, kind="ExternalInput") for m in "va"}

    d_full = {m: nc.dram_tensor(f"full_{m}", [NT, 2 * D], f32, kind="ExternalOutput") for m in "va"}
    d_vq = {m: nc.dram_tensor(f"vq_{m}", [NT, D], f32, kind="ExternalOutput") for m in "va"}
    d_idx = {m: nc.dram_tensor(f"idx_{m}", [NT, 1], u32, kind="ExternalOutput") for m in "va"}
    d_pH = {m: nc.dram_tensor(f"pH_{m}", [B // NCORES, M], f32, kind="ExternalOutput") for m in "va"}

    with TileContext(nc) as tc:
        with (
            tc.tile_pool(name="etab", bufs=1) as epool,     # eT splits + e2s bcast
            tc.tile_pool(name="work", bufs=2) as wpool,     # per-tile working tiles
            tc.tile_pool(name="ph", bufs=1) as hpool,       # pH accumulators
            tc.tile_pool(name="zbuf", bufs=1) as zpool,     # per-tile z/A kept for pH phase
            tc.tile_pool(name="sp", bufs=1) as spool,       # S' (single-buffered)
            tc.tile_pool(name="ps", bufs=8, space="PSUM") as ppool,
        ):
            for mod in "va":
                x_nat = d_video if mod == "v" else d_audio
                half = 0 if mod == "v" else D   # which half of the emb row for vq

                # --- modality tables: eT hi/lo [128, dc*2048], e2s broadcast ---
                eTh_t, eTl_t = [], []
                for dc in range(NDC):
                    th = epool.tile([128, M], f16, tag=f"eTh{dc}", name=f"eTh{dc}")
                    tl = epool.tile([128, M], f16, tag=f"eTl{dc}", name=f"eTl{dc}")
                    nc.sync.dma_start(th[:], d_eTh[mod][dc * 128 : (dc + 1) * 128, :])
                    nc.sync.dma_start(tl[:], d_eTl[mod][dc * 128 : (dc + 1) * 128, :])
                    eTh_t.append(th)
                    eTl_t.append(tl)
                e2sb = epool.tile([128, M], f32, tag="e2sb")
                nc.sync.dma_start(e2sb[:], d_e2s[mod][:].broadcast_to([128, M]))

                pH_sb = hpool.tile([B // NCORES, M], f32, tag="pH")
                z_tiles, A_tiles = [], []

                for tt in range(NTT):
                    trow = slice(tt * 128, (tt + 1) * 128)

                    # token-tile inputs: xT splits [128, dc*128]
                    xTh = wpool.tile([128, D], f16, tag="xTh")
                    xTl = wpool.tile([128, D], f16, tag="xTl")
                    nc.sync.dma_start(
                        xTh[:].rearrange("p (c t) -> p c t", c=NDC),
                        d_xTh[mod][:, trow].rearrange("(c p) t -> p c t", p=128))
                    nc.sync.dma_start(
                        xTl[:].rearrange("p (c t) -> p c t", c=NDC),
                        d_xTl[mod][:, trow].rearrange("(c p) t -> p c t", p=128))
                    x_t = wpool.tile([128, D], f32, tag="x")
                    nc.sync.dma_start(x_t[:], x_nat[trow, :])

                    # x2s = fl(sum(x^2)) * 2^21 (exact power-2 scale)
                    sqs = wpool.tile([128, D], f16, tag="sqscratch")
                    x2r = wpool.tile([128, 1], f32, tag="x2r")
                    nc.scalar.activation(sqs[:], x_t[:], AF.Square, accum_out=x2r[:])
                    x2s = wpool.tile([128, 1], f32, tag="x2s")
                    nc.vector.tensor_scalar_mul(x2s[:], x2r[:], S21)

                    # S' = fl(e2s + x2s)  (matches reference rounding x2+e2, scaled)
                    Sp = spool.tile([128, M], f32, tag="Sp")
                    nc.vector.tensor_scalar_add(Sp[:], e2sb[:], x2s[:])

                    # B~ = x' @ e'.T in 3 fp16 passes -> psum = 2B * 2^21
                    psB = [ppool.tile([128, 512], f32, tag="ps", name=f"psB{_mc}")
                           for _mc in range(NMC)]
                    for dc in range(NDC):
                        for pi, (xs_t, es_t) in enumerate(
                                ((xTh, eTh_t[dc]), (xTh, eTl_t[dc]), (xTl, eTh_t[dc]))):
                            lhs = xs_t[:, dc * 128 : (dc + 1) * 128]
                            for mc in range(NMC):
                                nc.tensor.matmul(
                                    psB[mc][:], lhs,
                                    es_t[:, mc * 512 : (mc + 1) * 512],
                                    start=(dc == 0 and pi == 0),
                                    stop=(dc == NDC - 1 and pi == 2),
                                )

                    # nd = fl(B~ - S') = -dist * 2^21 elementwise
                    nd = wpool.tile([128, M], f32, tag="nd")
                    for mc in range(NMC):
                        nc.vector.tensor_tensor(
                            out=nd[:, mc * 512 : (mc + 1) * 512], in0=psB[mc][:],
                            in1=Sp[:, mc * 512 : (mc + 1) * 512], op=OP.subtract)

                    # argmax(nd) with first-index tie-break == argmin(dist)
                    m8 = wpool.tile([128, 8], f32, tag="m8")
                    i8 = wpool.tile([128, 8], u32, tag="i8")
                    nc.vector.max_with_indices(m8[:], i8[:], nd[:])
                    i8f = wpool.tile([128, 8], f32, tag="i8f")
                    nc.vector.tensor_copy(i8f[:], i8[:])
                    maskf = wpool.tile([128, 8], f32, tag="maskf")
                    nc.vector.tensor_scalar(maskf[:], m8[:], m8[:, 0:1], None, op0=OP.is_equal)
                    nm = wpool.tile([128, 8], f32, tag="nm")
                    nc.vector.tensor_scalar(nm[:], maskf[:], -4096.0, 4096.0, op0=OP.mult, op1=OP.add)
                    nc.vector.tensor_tensor(out=i8f[:], in0=i8f[:], in1=nm[:], op=OP.add)
                    vidxf = wpool.tile([128, 1], f32, tag="vidxf")
                    nc.vector.tensor_reduce(vidxf[:], i8f[:], axis=mybir.AxisListType.X, op=OP.min)
                    vidx = wpool.tile([128, 1], u32, tag="vidx")
                    nc.vector.tensor_copy(vidx[:], vidxf[:])
                    nc.scalar.dma_start(d_idx[mod][trow, :], vidx[:])

                    # sqrt(dist) and softmax numerator/denominator on ACT
                    bias_s = wpool.tile([128, 1], f32, tag="bias")
                    nc.scalar.activation(bias_s[:], m8[:, 0:1], AF.Sqrt, scale=-float(2.0 ** -21))
                    sq = wpool.tile([128, M], f32, tag="sq")
                    nc.scalar.activation(sq[:], nd[:], AF.Sqrt, scale=-float(2.0 ** -21))
                    z = zpool.tile([128, M], f16, tag=f"z{tt}")
                    zsum = wpool.tile([128, 1], f32, tag="zsum")
                    nc.scalar.activation(z[:], sq[:], AF.Exp, bias=bias_s[:], scale=-1.0,
                                         accum_out=zsum[:])
                    z_tiles.append(z)

                    # A[t, b] = 1/(64*zsum_t) on this tile's 2 batches (for deferred pH)
                    rcp = wpool.tile([128, 1], f32, tag="rcp")
                    nc.vector.reciprocal(rcp[:], zsum[:])
                    rs = wpool.tile([128, 1], f32, tag="rs")
                    nc.vector.tensor_scalar_mul(rs[:], rcp[:], float(2.0 ** -6))
                    A_t = zpool.tile([128, 16], f16, tag=f"A{tt}")
                    nc.vector.memset(A_t[:], 0.0)
                    nc.vector.tensor_copy(A_t[0:64, 2 * tt : 2 * tt + 1], rs[0:64, :])
                    nc.vector.tensor_copy(A_t[64:128, 2 * tt + 1 : 2 * tt + 2], rs[64:128, :])
                    A_tiles.append(A_t)

                    # gather full codebook rows; straight-through vq in ref op order
                    g = wpool.tile([128, 2 * D], f32, tag="g")
                    nc.gpsimd.indirect_dma_start(
                        out=g[:], out_offset=None, in_=d_emb[:],
                        in_offset=bass.IndirectOffsetOnAxis(ap=vidx[:, :1], axis=0))
                    vq = wpool.tile([128, D], f32, tag="vq")
                    nc.vector.tensor_tensor(out=vq[:], in0=g[:, half : half + D], in1=x_t[:], op=OP.subtract)
                    nc.vector.tensor_tensor(out=vq[:], in0=vq[:], in1=x_t[:], op=OP.add)
                    nc.scalar.dma_start(d_full[mod][trow, :], g[:])
                    nc.scalar.dma_start(d_vq[mod][trow, :], vq[:])

                for mc in range(NMC):
                    psP = ppool.tile([16, 512], f32, tag="ps", name=f"psP{mc}")
                    for tt in range(NTT):
                        nc.tensor.matmul(psP[:], A_tiles[tt][:],
                                         z_tiles[tt][:, mc * 512 : (mc + 1) * 512],
                                         start=(tt == 0), stop=(tt == NTT - 1))
                    nc.vector.tensor_copy(pH_sb[:, mc * 512 : (mc + 1) * 512], psP[:])
                nc.scalar.dma_start(d_pH[mod][:], pH_sb[:])

    _split_waits(nc, max_waits=1)
    return nc


def _prep_host(audio, video, embedding):
    """Host-side prep: per-core slices, scaled fp16 hi/lo splits (transposed),
    e2 row sums."""
    a = np.ascontiguousarray(np.asarray(audio, dtype=np.float32).reshape(BT, D))
    v = np.ascontiguousarray(np.asarray(video, dtype=np.float32).reshape(BT, D))
    emb = np.ascontiguousarray(np.asarray(embedding, dtype=np.float32))

    def split_T(x, s):
        xs = x * s
        h = xs.astype(np.float16)
        l = (xs - h.astype(np.float32)).astype(np.float16)
        return np.ascontiguousarray(h.T), np.ascontiguousarray(l.T)

    vTh, vTl = split_T(v, XS)       # [D, BT]
    aTh, aTl = split_T(a, XS)
    e_v, e_a = emb[:, :D], emb[:, D:]
    evTh, evTl = split_T(e_v, ES)   # [D, M]
    eaTh, eaTl = split_T(e_a, ES)
    e2s_v = ((e_v * e_v).sum(axis=1, dtype=np.float32).astype(np.float32)
             * np.float32(S21)).reshape(1, M)
    e2s_a = ((e_a * e_a).sum(axis=1, dtype=np.float32).astype(np.float32)
             * np.float32(S21)).reshape(1, M)

    in_maps = []
    for c in range(NCORES):
        r = slice(c * NT, (c + 1) * NT)
        in_maps.append({
            "video": v[r], "audio": a[r], "emb": emb,
            "xTh_v": np.ascontiguousarray(vTh[:, r]),
            "xTl_v": np.ascontiguousarray(vTl[:, r]),
            "xTh_a": np.ascontiguousarray(aTh[:, r]),
            "xTl_a": np.ascontiguousarray(aTl[:, r]),
            "eTh_v": evTh, "eTl_v": evTl, "eTh_a": eaTh, "eTl_a": eaTl,
            "e2s_v": e2s_v, "e2s_a": e2s_a,
        })
    return in_maps, (a, v, emb)


def _finish_host(results, a, v, emb):
    """Assemble per-core outputs; compute Lcmcm + equal_num on host (fp32)."""
    full_v = np.concatenate([r["full_v"] for r in results], axis=0)
    full_a = np.concatenate([r["full_a"] for r in results], axis=0)
    vq_v = np.concatenate([r["vq_v"] for r in results], axis=0)
    vq_a = np.concatenate([r["vq_a"] for r in results], axis=0)
    idx_v = np.concatenate([r["idx_v"] for r in results], axis=0)[:, 0].astype(np.int64)
    idx_a = np.concatenate([r["idx_a"] for r in results], axis=0)[:, 0].astype(np.int64)
    pH_v = np.concatenate([r["pH_v"] for r in results], axis=0)  # [B, M]
    pH_a = np.concatenate([r["pH_a"] for r in results], axis=0)

    eps = np.float32(1e-5)
    logv = np.log(pH_v.T + np.float32(1e-10), dtype=np.float32)
    loga = np.log(pH_a.T + np.float32(1e-10), dtype=np.float32)
    Scode = pH_a @ logv + pH_v @ loga
    MaxScode = np.max(-Scode)
    EScode = np.exp(Scode + MaxScode, dtype=np.float32)
    ES1 = EScode.sum(axis=1, dtype=np.float32)
    Lcmcm = np.float32(-np.mean(np.log(np.diagonal(EScode) / (ES1 + eps),
                                       dtype=np.float32), dtype=np.float32))

    iv = idx_v.reshape(B, T)
    ia = idx_a.reshape(B, T)
    cv = np.zeros((B, M), np.int32)
    ca = np.zeros((B, M), np.int32)
    rows = np.repeat(np.arange(B), T)
    np.add.at(cv, (rows, iv.reshape(-1)), 1)
    np.add.at(ca, (rows, ia.reshape(-1)), 1)
    equal_num = np.int32((cv.argmax(axis=1) == ca.argmax(axis=1)).sum())

    return (
        full_v.reshape(B, T, 2 * D), full_a.reshape(B, T, 2 * D),
        vq_v.reshape(B, T, D), vq_a.reshape(B, T, D),
        Lcmcm, equal_num,
    )


def kernel(audio_semantic, video_semantic, embedding, modality_weights,
           hierarchical_weights, epoch):
    from concourse.bass_utils import run_bass_kernel_spmd

    if "nc" not in _CACHE:
        _CACHE["nc"] = _build()
    nc = _CACHE["nc"]

    in_maps, (a, v, emb) = _prep_host(audio_semantic, video_semantic, embedding)
    res = run_bass_kernel_spmd(nc, in_maps, core_ids=list(range(NCORES)))
    return _finish_host(res.results, a, v, emb)
